# revision 1
# baseline (speedup 1.0000x reference)
"""Trainium2 Bass kernel for nn_MixUniformAffineQuantizer.

kernel(x, upbound_factor, lowbound_factor) -> [4096, 11008] f32.

Strategy: rows sharded 512/core across 8 NeuronCores (row-parallel, no
communication). Per core, per [128-row x 5504-col] chunk:
  - DVE 3D-view tensor_reduce: per-group min/max (+ sum/|sum| for the
    ternary group 0)
  - per-group scale / zero-point math on [128, 43] stat tiles, replicating
    the jax/XLA eager reference BITWISE (XLA lowers f32 divide as
    multiply-by-reciprocal; exp2 via exp -> levels of 0.99999833, 3,
    6.999998, 14.999921; round-half-even realized with the 1.5*2^23
    magic-number add/sub)
  - fake-quant chain per group g: ACT computes round(x*(1/s)) + M in one
    Identity activation (bias M lands the sum on the fp32 integer grid),
    then DVE tensor_scalar instructions clip and dequantize in the scaled
    domain: y = clip(r*s, (-z)*s, (qmaxJ-z)*s), which is bitwise equal to
    the reference's (clip(r+z, 0, qmaxJ) - z) * s by monotonicity.
  - sigmoid(upbound/lowbound) is computed host-side with jax (default
    device) and fed as inputs, matching the reference bitwise.
"""
import sys
import numpy as np

for _p in ("/opt/trn_rl_repo", "/root/.axon_site/_ro/trn_rl_repo"):
    if _p not in sys.path:
        sys.path.append(_p)

from contextlib import ExitStack
import concourse.bass as bass
import concourse.tile as tile
from concourse import bacc, mybir
from concourse.bass_utils import run_bass_kernel_spmd

F32 = mybir.dt.float32
ALU = mybir.AluOpType
ACTF = mybir.ActivationFunctionType

ROWS, COLS, G, NB = 4096, 11008, 128, 86
NCORES = 8
R = ROWS // NCORES    # 512 rows per core
NCH = 2               # col chunks per row-tile
GCH = NB // NCH       # 43 groups per chunk
CH = GCH * G          # 5504 cols per chunk
M = 12582912.0        # 1.5*2^23 round-to-even magic
CLIPMIN, CLIPMAX = 1e-5, 1e4

_PREC = np.array([1] + [2, 3, 4, 3, 2] * 17, dtype=np.int32)

CHAIN = "fullwidth"   # "fullwidth" | "groupwise"
LOOKAHEAD = 2

_LEVELS = None


def _levels_jax():
    """2^p - 1 exactly as the jax reference computes it (default device)."""
    global _LEVELS
    if _LEVELS is None:
        import jax.numpy as jnp
        _LEVELS = np.asarray(
            jnp.exp2(jnp.asarray(_PREC).astype(jnp.float32)) - 1.0
        ).astype(np.float32)
    return _LEVELS


def _bv(small_ap, width=G):
    """[128, n] AP -> [128, n, width] stride-0 broadcast view."""
    return bass.AP(small_ap.tensor, small_ap.offset,
                   [small_ap.ap[0], small_ap.ap[1], [0, width]])


def _build(nc):
    lvj = _levels_jax()
    q1 = float(lvj[0])  # ternary clip-high (~0.99999833)
    x = nc.dram_tensor("x", [R, COLS], F32, kind="ExternalInput").ap()
    su = nc.dram_tensor("su", [R, NB], F32, kind="ExternalInput").ap()
    sl = nc.dram_tensor("sl", [R, NB], F32, kind="ExternalInput").ap()
    su05 = nc.dram_tensor("su05", [R, 1], F32, kind="ExternalInput").ap()
    ilv = nc.dram_tensor("inv_levels", [128, NB], F32, kind="ExternalInput").ap()
    lv = nc.dram_tensor("levels", [128, NB], F32, kind="ExternalInput").ap()
    out = nc.dram_tensor("out", [R, COLS], F32, kind="ExternalOutput").ap()
    NT = R // 128

    with tile.TileContext(nc) as tc, ExitStack() as ctx:
        cpool = ctx.enter_context(tc.tile_pool(name="const", bufs=1))
        xpool = ctx.enter_context(tc.tile_pool(name="xp", bufs=LOOKAHEAD + 2))
        ypool = ctx.enter_context(tc.tile_pool(name="yp", bufs=2))
        vpool = ctx.enter_context(tc.tile_pool(name="vp", bufs=2))
        rpool = ctx.enter_context(tc.tile_pool(name="rowp", bufs=2))
        spool = ctx.enter_context(tc.tile_pool(name="statp", bufs=LOOKAHEAD + 1))
        gpool = ctx.enter_context(tc.tile_pool(name="gp", bufs=8))

        lv_t = cpool.tile([128, NB], F32, tag="lv")
        nc.sync.dma_start(lv_t[:], lv[:])
        ilv_t = cpool.tile([128, NB], F32, tag="ilv")
        nc.sync.dma_start(ilv_t[:], ilv[:])
        Mb = cpool.tile([128, 1], F32, tag="Mb")
        nc.vector.memset(Mb[:], M)

        chunks = [(rt, c) for rt in range(NT) for c in range(NCH)]
        n = len(chunks)
        state = {}
        rowstate = {}

        def stage_front(k):
            rt, c = chunks[k]
            if c == 0:
                sut = rpool.tile([128, NB], F32, tag="su")
                nc.sync.dma_start(sut[:], su[rt * 128:(rt + 1) * 128, :])
                slt = rpool.tile([128, NB], F32, tag="sl")
                nc.sync.dma_start(slt[:], sl[rt * 128:(rt + 1) * 128, :])
                s5t = rpool.tile([128, 1], F32, tag="su05")
                nc.sync.dma_start(s5t[:], su05[rt * 128:(rt + 1) * 128, :])
                rowstate[rt] = (sut, slt, s5t)
            sut, slt, s5t = rowstate[rt]

            xt = xpool.tile([128, CH], F32, tag="x")
            for q in range(4):
                nc.sync.dma_start(
                    xt[q * 32:(q + 1) * 32, :],
                    x[rt * 128 + q * 32:rt * 128 + (q + 1) * 32,
                      c * CH:(c + 1) * CH])

            gsl = slice(c * GCH, (c + 1) * GCH)
            xv = xt[:, :].rearrange("p (g j) -> p g j", j=G)
            rmin = spool.tile([128, GCH], F32, tag="rmin")
            rmax = spool.tile([128, GCH], F32, tag="rmax")
            nc.vector.tensor_reduce(rmin[:], xv, axis=mybir.AxisListType.X, op=ALU.min)
            nc.vector.tensor_reduce(rmax[:], xv, axis=mybir.AxisListType.X, op=ALU.max)

            xsmax = spool.tile([128, GCH], F32, tag="xsmax")
            nc.vector.tensor_tensor(xsmax[:], sut[:, gsl], rmax[:], op=ALU.mult)
            xsmin = spool.tile([128, GCH], F32, tag="xsmin")
            nc.vector.tensor_tensor(xsmin[:], slt[:, gsl], rmin[:], op=ALU.mult)
            diff = spool.tile([128, GCH], F32, tag="diff")
            nc.vector.tensor_tensor(diff[:], xsmax[:], xsmin[:], op=ALU.subtract)
            scale_r = spool.tile([128, GCH], F32, tag="scale_r")
            nc.vector.tensor_tensor(scale_r[:], diff[:], ilv_t[:, gsl], op=ALU.mult)
            rcp = spool.tile([128, GCH], F32, tag="rcp")
            nc.vector.reciprocal(rcp[:], scale_r[:])
            t1 = spool.tile([128, GCH], F32, tag="t1")
            nc.vector.tensor_tensor(t1[:], xsmin[:], rcp[:], op=ALU.mult)
            t2 = spool.tile([128, GCH], F32, tag="t2")
            nc.vector.tensor_scalar(t2[:], t1[:], -CLIPMAX, CLIPMAX, op0=ALU.max, op1=ALU.min)
            t3 = spool.tile([128, GCH], F32, tag="t3")
            nc.vector.tensor_scalar(t3[:], t2[:], M, M, op0=ALU.add, op1=ALU.subtract)
            scl = spool.tile([128, GCH], F32, tag="scl")
            nc.vector.tensor_scalar(scl[:], scale_r[:], CLIPMIN, CLIPMAX, op0=ALU.max, op1=ALU.min)
            rs = spool.tile([128, GCH], F32, tag="rs")
            nc.vector.reciprocal(rs[:], scl[:])

            st = {"xt": xt, "rs": rs, "scl": scl}
            if CHAIN == "groupwise":
                Mnz = spool.tile([128, GCH], F32, tag="Mnz")
                nc.vector.tensor_scalar(Mnz[:], t3[:], M, None, op0=ALU.add)
                QZ = spool.tile([128, GCH], F32, tag="QZ")
                nc.vector.tensor_tensor(QZ[:], t3[:], lv_t[:, gsl], op=ALU.add)
                st["Mnz"], st["QZ"] = Mnz, QZ
            else:
                QZ = spool.tile([128, GCH], F32, tag="QZ")
                nc.vector.tensor_tensor(QZ[:], t3[:], lv_t[:, gsl], op=ALU.add)
                NZS = spool.tile([128, GCH], F32, tag="NZS")
                nc.vector.tensor_tensor(NZS[:], t3[:], scl[:], op=ALU.mult)
                QZS = spool.tile([128, GCH], F32, tag="QZS")
                nc.vector.tensor_tensor(QZS[:], QZ[:], scl[:], op=ALU.mult)
                st["NZS"], st["QZS"] = NZS, QZS
            if c == 0:
                x0v = xt[:, 0:G].rearrange("p (g j) -> p g j", j=G)
                rsum = spool.tile([128, 1], F32, tag="rsum")
                nc.vector.tensor_reduce(rsum[:], x0v, axis=mybir.AxisListType.X, op=ALU.add)
                rabs = spool.tile([128, 1], F32, tag="rabs")
                nc.vector.tensor_reduce(rabs[:], x0v, axis=mybir.AxisListType.X, op=ALU.add,
                                        apply_absolute_value=True)
                nzt_a = spool.tile([128, 1], F32, tag="nzt_a")
                nc.vector.tensor_scalar(nzt_a[:], rsum[:], -1.0 / 128.0, -CLIPMAX,
                                        op0=ALU.mult, op1=ALU.max)
                nzt = spool.tile([128, 1], F32, tag="nzt")
                nc.vector.tensor_scalar(nzt[:], nzt_a[:], CLIPMAX, None, op0=ALU.min)
                sta = spool.tile([128, 1], F32, tag="sta")
                nc.vector.tensor_scalar(sta[:], rabs[:], 1.0 / 128.0, s5t[:],
                                        op0=ALU.mult, op1=ALU.mult)
                stt = spool.tile([128, 1], F32, tag="stt")
                nc.vector.tensor_scalar(stt[:], sta[:], CLIPMIN, CLIPMAX,
                                        op0=ALU.max, op1=ALU.min)
                st["nzt"] = nzt
                st["stt"] = stt
            state[k] = st

        def ternary_ops(st, xt, yt):
            v0 = gpool.tile([128, G], F32, tag="v0")
            nc.scalar.sign(v0[:], xt[:, 0:G], bias=st["nzt"][:])
            nc.vector.tensor_scalar(yt[:, 0:G], v0[:], q1, st["stt"][:],
                                    op0=ALU.min, op1=ALU.mult)

        def stage_back_groupwise(k):
            rt, c = chunks[k]
            st = state.pop(k)
            xt, rs, Mnz, QZ, scl = st["xt"], st["rs"], st["Mnz"], st["QZ"], st["scl"]
            yt = ypool.tile([128, CH], F32, tag="y")
            g0 = 0
            if c == 0:
                ternary_ops(st, xt, yt)
                g0 = 1
            for g in range(g0, GCH):
                v = gpool.tile([128, G], F32, tag="v")
                nc.scalar.activation(v[:], xt[:, g * G:(g + 1) * G], ACTF.Identity,
                                     bias=Mb[:], scale=rs[:, g:g + 1])
                nc.vector.tensor_scalar(v[:], v[:], Mnz[:, g:g + 1], M,
                                        op0=ALU.max, op1=ALU.subtract)
                nc.vector.tensor_scalar(yt[:, g * G:(g + 1) * G], v[:], QZ[:, g:g + 1],
                                        scl[:, g:g + 1], op0=ALU.min, op1=ALU.mult)
            nc.sync.dma_start(out[rt * 128:(rt + 1) * 128, c * CH:(c + 1) * CH], yt[:])

        def stage_back_fullwidth(k):
            rt, c = chunks[k]
            st = state.pop(k)
            xt, rs, scl = st["xt"], st["rs"], st["scl"]
            vt = vpool.tile([128, CH], F32, tag="vch")
            for g in range(GCH):
                nc.scalar.activation(vt[:, g * G:(g + 1) * G], xt[:, g * G:(g + 1) * G],
                                     ACTF.Identity, bias=Mb[:], scale=rs[:, g:g + 1])
            yt = ypool.tile([128, CH], F32, tag="y")
            vv = vt[:, :].rearrange("p (g j) -> p g j", j=G)
            yv = yt[:, :].rearrange("p (g j) -> p g j", j=G)
            nc.vector.scalar_tensor_tensor(yv, vv, M, _bv(scl[:, :]),
                                           op0=ALU.subtract, op1=ALU.mult)
            nc.vector.tensor_tensor(yv, yv, _bv(st["NZS"][:, :]), op=ALU.max)
            nc.vector.tensor_tensor(yv, yv, _bv(st["QZS"][:, :]), op=ALU.min)
            if c == 0:
                ternary_ops(st, xt, yt)
            nc.sync.dma_start(out[rt * 128:(rt + 1) * 128, c * CH:(c + 1) * CH], yt[:])

        stage_back = stage_back_fullwidth if CHAIN == "fullwidth" else stage_back_groupwise
        for k in range(n + LOOKAHEAD):
            if k < n:
                stage_front(k)
            if k >= LOOKAHEAD:
                stage_back(k - LOOKAHEAD)
    return nc


_COMPILED = None


def _get_compiled():
    global _COMPILED
    if _COMPILED is None:
        nc = bacc.Bacc("TRN2", target_bir_lowering=False, debug=False)
        _build(nc)
        nc.compile()
        _COMPILED = nc
    return _COMPILED


def kernel(x, upbound_factor, lowbound_factor):
    import jax, jax.numpy as jnp
    x = np.ascontiguousarray(np.asarray(x, dtype=np.float32))
    up = np.asarray(upbound_factor, dtype=np.float32)
    low = np.asarray(lowbound_factor, dtype=np.float32)
    assert x.shape == (ROWS, COLS) and up.shape == (ROWS, NB) and low.shape == (ROWS, NB)

    # host precompute (matches the reference's own jax ops bitwise)
    su = np.asarray(jax.nn.sigmoid(jnp.asarray(up))).astype(np.float32)
    sl = np.asarray(jax.nn.sigmoid(jnp.asarray(low))).astype(np.float32)
    su05 = (su[:, 0:1] + np.float32(0.5)).astype(np.float32)
    lvj = _levels_jax()
    lv = np.ascontiguousarray(np.broadcast_to(lvj[None, :], (128, NB)), dtype=np.float32)
    ilv = np.ascontiguousarray(
        np.broadcast_to((np.float32(1.0) / lvj)[None, :], (128, NB)), dtype=np.float32)

    in_maps = []
    for i in range(NCORES):
        r0, r1 = i * R, (i + 1) * R
        in_maps.append({
            "x": np.ascontiguousarray(x[r0:r1]),
            "su": np.ascontiguousarray(su[r0:r1]),
            "sl": np.ascontiguousarray(sl[r0:r1]),
            "su05": np.ascontiguousarray(su05[r0:r1]),
            "inv_levels": ilv,
            "levels": lv,
        })

    nc = _get_compiled()
    res = run_bass_kernel_spmd(nc, in_maps, core_ids=list(range(NCORES)), trace=False)
    return np.concatenate([np.asarray(res.results[i]["out"], dtype=np.float32)
                           for i in range(NCORES)], axis=0)

